# revision 29
# baseline (speedup 1.0000x reference)
"""Trainium2 Bass kernel for multi-head quadratic spatial attention.

Problem: q,k,v [b=8, heads=8, h=32, w=32, d=64] fp32; full attention over
the 1024-position spatial grid independently per (b, head); output
[b, h, w, heads*d].

Sharding: data-parallel over batch — core c handles b=c (8 heads of
[1024, 64] attention per core), no cross-core communication.

Per-core pipeline (heads processed in PAIRS; matmuls bf16 with fp32 PSUM
accumulation). The PE executes serially on this toolchain, so the design
minimizes streamed columns + instruction count and keeps the HAM clock
gate warm (no transpose-heavy stretches > ~3.4us, dummy-matmul warm-up):
  - p-major seq tiling (seq = p*8 + t); ONE 4D casting DMA per (tensor,
    pair) interleaving the two heads -> 3 gpsimd triggers per pair
  - 40 dummy ident matmuls warm the PE clock gate (1.2 -> 2.4 GHz) while
    the first DMAs land
  - pair-interleaved natural tiles [128, t, 2, d]: one [128,128] PE
    transpose per block yields head A's d-rows on partitions 0:64 and
    B's on 64:128 — the packed pair layout mm1 wants
  - mm1 row-tiled: head A contracts on PE rows 0:64, head B on 64:128
    -> St [128, 1024] fp32 (separate tiles, freed by their own exp)
  - exp on ScalarE (activation Exp); optional per-(jb, head) offload to
    VectorE via the Schraudolph bit-trick (fused tensor_scalar
    mult+add -> int16 == bf16 exp approx) to unload the ScalarE
  - mm2: lhsT = [V | 1] j-chunk [128, 65] bf16, rhs = Pt slices ->
    accumulate PSUM Ot [65, 512] per i-half; row 64 = softmax sums
  - epilogue in bf16: ot copy on VectorE, PE transposes back (FWL), one
    batched reciprocal [128,4] + per-block tensor_scalar normalize into
    fp32 ostage, stores on the sync HWDGE ring
"""

from contextlib import ExitStack

import numpy as np

F32 = None
BF16 = None
I16 = None

_cache = {}

# Schraudolph exp in bf16 bit-space: bf16_bits(exp(s*x)) ~= round(x*A + B)
# A = s * 2^7/ln2, B = 2^7*(127 - sigma), sigma = 0.0430 balances the
# piecewise-linear 2^frac error (max rel err ~3%, mostly cancelling in the
# softmax average).
SCALE = 64.0 ** -0.5
SCHRAUD_A = SCALE * 128.0 / float(np.log(2.0))
SCHRAUD_B = 128.0 * (127.0 - 0.0430)

N_WARM = 40  # dummy matmuls to flip the PE HAM clock gate before real work

# (jb, head-in-pair) St tiles exp'd on VectorE via the Schraudolph bit-trick
# instead of ScalarE's exact exp. Each entry moves 1/16 of the attention
# weight mass to a ~3% weight-error approximation (mostly cancelling in the
# softmax average) and takes ~1.1us/pair off the ScalarE critical path.
SCHRAUD_TILES = frozenset()


def _imports():
    global F32, BF16, I16
    import concourse.bass as bass
    import concourse.tile as tile
    from concourse import mybir
    from concourse.masks import make_identity

    F32 = mybir.dt.float32
    BF16 = mybir.dt.bfloat16
    I16 = mybir.dt.int16
    return bass, tile, mybir, make_identity


def _split_multi_waits(nc, mybir):
    """Walrus in this container supports only ONE sync-wait per instruction.
    Hoist extra waits onto same-engine InstNoOp's inserted just before."""
    ctr = 0
    for f in nc.m.functions:
        for bb in f.blocks:
            insts = bb.instructions
            if not any(
                i.sync_info and i.sync_info.on_wait and len(i.sync_info.on_wait) > 1
                for i in insts
            ):
                continue
            out = []
            for inst in insts:
                si = inst.sync_info
                waits = list(si.on_wait) if si and si.on_wait else []
                if len(waits) > 1:
                    for w in waits[:-1]:
                        ctr += 1
                        nop = mybir.InstNoOp(
                            name=f"I-wsplit-{ctr}",
                            engine=inst.engine,
                            ins=[],
                            outs=[],
                            sync_info=mybir.SyncInfo(on_wait=[w], on_update=[]),
                        )
                        nc.register_instruction(nop)
                        out.append(nop)
                    si.on_wait = waits[-1:]
                out.append(inst)
            bb.instructions = out


def _build_nc(heads=8, seq=1024, d=64):
    bass, tile, mybir, make_identity = _imports()
    assert heads % 2 == 0 and seq == 1024 and d == 64
    nt = seq // 128          # 8 blocks of 128 positions
    nh = seq // 512          # 2 i-halves of 512
    dv = d + 1
    TS_MULT = mybir.AluOpType.mult
    TS_ADD = mybir.AluOpType.add

    nc = bass.Bass(trn_type="TRN2", target_bir_lowering=False)
    q_d = nc.dram_tensor("q", [heads, seq, d], F32, kind="ExternalInput")
    k_d = nc.dram_tensor("k", [heads, seq, d], F32, kind="ExternalInput")
    v_d = nc.dram_tensor("v", [heads, seq, d], F32, kind="ExternalInput")
    o_d = nc.dram_tensor("out", [seq, heads * d], F32, kind="ExternalOutput")

    # p-major: seq = p*nt + t; per-(p, t) HBM runs are 256B contiguous
    q_ap = q_d[:].rearrange("n (p t) d -> n p t d", p=128)
    k_ap = k_d[:].rearrange("n (p t) d -> n p t d", p=128)
    v_ap = v_d[:].rearrange("n (p t) d -> n p t d", p=128)
    o_ap = o_d[:].rearrange("(p t) c -> p t c", p=128)

    with tile.TileContext(nc) as tc, ExitStack() as ctx:
        consts = ctx.enter_context(tc.tile_pool(name="consts", bufs=1))
        nat = ctx.enter_context(tc.tile_pool(name="nat", bufs=2))
        dmaj = ctx.enter_context(tc.tile_pool(name="dmaj", bufs=2))
        ptp = ctx.enter_context(tc.tile_pool(name="ptp", bufs=36))
        otp = ctx.enter_context(tc.tile_pool(name="otp", bufs=3))
        outp = ctx.enter_context(tc.tile_pool(name="outp", bufs=3))
        small = ctx.enter_context(tc.tile_pool(name="small", bufs=4))

        # PSUM banks: st 2x2 (0-3) + oacc/ob/warm 2x1 (4-5) + tp 2x1 (6-7)
        st_ps = ctx.enter_context(tc.tile_pool(name="st_ps", bufs=2, space="PSUM"))
        oa_ps = ctx.enter_context(tc.tile_pool(name="oa_ps", bufs=2, space="PSUM"))
        tp_ps = ctx.enter_context(tc.tile_pool(name="tp_ps", bufs=2, space="PSUM"))

        ident_bf = consts.tile([128, 128], BF16)
        make_identity(nc, ident_bf[:])

        # Warm-up / filler matmuls keep the PE HAM clock gate at 2.4 GHz:
        # an idle (or transpose-only) stretch > ~3.4us re-throttles the PE
        # clock to 1.2 GHz for the next several microseconds. wsrc is
        # memset-ready within ~200ns of kernel start. N=512 streams give
        # ~213ns of HAM-counted busy per filler instruction.
        wsrc = consts.tile([128, 512], BF16)
        nc.vector.memset(wsrc[:], 0.25)
        warm = oa_ps.tile([128, 512], F32, tag="oacc")

        def pe_filler(n):
            for _ in range(n):
                nc.tensor.matmul(
                    warm[:], wsrc[:, 0:128], wsrc[:], start=True, stop=True
                )

        pe_filler(N_WARM)

        def load_and_transpose(pair):
            """DMA pair inputs (bf16 cast, one 4D DMA per tensor) and build
            packed d-major tiles: head A on partitions 0:64, head B on
            64:128 (one [128,128] PE transpose per block)."""
            st8 = {"heads": (2 * pair, 2 * pair + 1), "v": None, "pts": [],
                   "oacc": {}, "ostage": {}}
            # pair-interleaved natural tiles: [..., 2, d] with head A at
            # index 0 and head B at 1, so one [128, 128] PE transpose of a
            # block yields A's d-rows on partitions 0:64 and B's on 64:128.
            qp = nat.tile([128, nt, 2, d], BF16, tag="qp")
            kp = nat.tile([128, nt, 2, d], BF16, tag="kp")
            hh = nt // 2
            for idx, n in enumerate(st8["heads"]):
                if pair == 0:
                    # halved loads so the first transposes start early
                    for lo, hi in ((0, hh), (hh, nt)):
                        nc.gpsimd.dma_start(
                            out=qp[:, lo:hi, idx, :], in_=q_ap[n, :, lo:hi]
                        )
                        nc.gpsimd.dma_start(
                            out=kp[:, lo:hi, idx, :], in_=k_ap[n, :, lo:hi]
                        )
                else:
                    nc.gpsimd.dma_start(out=qp[:, :, idx, :], in_=q_ap[n])
                    nc.gpsimd.dma_start(out=kp[:, :, idx, :], in_=k_ap[n])
            vp = nat.tile([128, nt, 2, dv], BF16, tag="vp")
            # ones columns for the softmax-denominator trick
            nc.vector.memset(vp[:, :, :, d : d + 1], 1.0)
            for idx, n in enumerate(st8["heads"]):
                nc.gpsimd.dma_start(out=vp[:, :, idx, 0:d], in_=v_ap[n])
            st8["v"] = vp
            qt = dmaj.tile([128, seq], BF16, tag="qt")
            kt = dmaj.tile([128, seq], BF16, tag="kt")
            for g in range(nt // 4):
                for src, dst in ((qp, qt), (kp, kt)):
                    tp = tp_ps.tile([128, 512], BF16, tag="tp")
                    for u in range(4):
                        t = g * 4 + u
                        nc.tensor.transpose(
                            tp[:, u * 128 : (u + 1) * 128],
                            src[:, t, :, :],
                            ident_bf[:],
                        )
                    nc.vector.tensor_copy(
                        out=dst[:, g * 512 : (g + 1) * 512], in_=tp[:]
                    )
                    if pair == 0:
                        # PE is otherwise DMA-bound here; keep the clock warm
                        pe_filler(3)
            st8["qt"], st8["kt"] = qt, kt
            return st8

        def mm1_exp(s, jb):
            """Row-tiled pair mm1 into per-head St tiles + per-head exp.
            Separate St tiles mean head A's tile is released as soon as its
            own exp finishes."""
            qt, kt = s["qt"], s["kt"]
            stA = st_ps.tile([128, seq], F32, name="stA", tag="st")
            stB = st_ps.tile([128, seq], F32, name="stB", tag="st")
            for c in range(nh):
                nc.tensor.matmul(
                    stA[:, c * 512 : (c + 1) * 512],
                    kt[0:64, jb * 128 : (jb + 1) * 128],
                    qt[0:64, c * 512 : (c + 1) * 512],
                    start=True,
                    stop=True,
                )
                nc.tensor.matmul(
                    stB[:, c * 512 : (c + 1) * 512],
                    kt[64:128, jb * 128 : (jb + 1) * 128],
                    qt[64:128, c * 512 : (c + 1) * 512],
                    start=True,
                    stop=True,
                )
            for idx, st in enumerate((stA, stB)):
                pt = ptp.tile([128, seq], BF16, name="pt", tag="pt")
                if (jb, idx) in SCHRAUD_TILES:
                    nc.vector.tensor_scalar(
                        out=pt[:].bitcast(I16),
                        in0=st[:],
                        scalar1=SCHRAUD_A,
                        scalar2=SCHRAUD_B,
                        op0=TS_MULT,
                        op1=TS_ADD,
                    )
                else:
                    nc.scalar.activation(
                        out=pt[:],
                        in_=st[:],
                        func=mybir.ActivationFunctionType.Exp,
                        scale=SCALE,
                    )
                s["pts"].append((jb, idx, pt))

        # slot s -> (group, phase); phase-1 slots sit at s>=2 so they only
        # run once all 8 jbs' Pt tiles exist (6-jb-shifted pipeline), while
        # at most 2 groups' oacc accumulators are ever live.
        SLOT_ORDER = [(0, 0), (1, 0), (0, 1), (1, 1), (2, 0), (3, 0), (2, 1), (3, 1)]

        def mm2_slot(s, slot, tail=False):
            """One PE-stream slot of the pair's mm2: 4 accumulating
            matmuls of a (head, half) group; epilogue on the closing
            phase."""
            g, phase = SLOT_ORDER[slot]
            idx, half = g // 2, g % 2
            if phase == 0:
                s["oacc"][g] = oa_ps.tile([dv, 512], F32, name="oacc", tag="oacc")
            oacc = s["oacc"][g]
            off = half * 512
            for jj in range(4):
                jb = phase * 4 + jj
                jb2, idx2, pt = s["pts"][jb * 2 + idx]
                assert jb2 == jb and idx2 == idx
                nc.tensor.matmul(
                    oacc[:],
                    s["v"][:, jb, idx, :],
                    pt[:, off : off + 512],
                    start=(jb == 0),
                    stop=(jb == nt - 1),
                )
            if phase == 1:
                _epilogue(s, idx, half, oacc, tail=tail)

        def _epilogue(s, idx, half, oacc, tail=False):
            n = s["heads"][idx]
            if idx not in s["ostage"]:
                s["ostage"][idx] = outp.tile(
                    [128, nt, d], F32, name="ostage", tag="ostage"
                )
            ostage = s["ostage"][idx]
            ot = otp.tile([dv, 512], BF16, tag="ot")
            if tail:
                # ScalarE is idle after its last exp; shorten the tail chain
                nc.scalar.copy(out=ot[:], in_=oacc[:])
            else:
                nc.vector.tensor_copy(out=ot[:], in_=oacc[:])
            # ob shares the oacc pool banks: rotation interleaves
            # oacc(g) -> ob(g) -> oacc(g+1), each WAR-safe by then.
            # dv+1 padding keeps each transpose's PSUM write 4B-aligned.
            ob = oa_ps.tile([128, 4, dv + 1], BF16, tag="oacc")
            for u in range(4):
                nc.tensor.transpose(
                    ob[:, u, 0:dv],
                    ot[:, u * 128 : (u + 1) * 128],
                    ident_bf[0:dv, 0:dv],
                )
            rec = small.tile([128, 4], F32, tag="rec")
            nc.vector.reciprocal(out=rec[:], in_=ob[:, :, d])
            for u in range(4):
                t = half * 4 + u
                nc.vector.tensor_scalar_mul(
                    ostage[:, t, :], ob[:, u, 0:d], rec[:, u : u + 1]
                )
            if half == nh - 1:
                nc.sync.dma_start(
                    out=o_ap[:, :, n * d : (n + 1) * d], in_=ostage[:]
                )

        # software pipeline, 6-jb shifted: mm2 slot S runs alongside mm1 of
        # global jb S+6, so pair 0's loop is mm2-dense from jb 6 (instead
        # of mm1-only for a whole pair) and the mm2-only tail is 6 slots.
        SHIFT = 6
        states = []
        for pair in range(heads // 2):
            cur = load_and_transpose(pair)
            states.append(cur)
            for jb in range(nt):
                mm1_exp(cur, jb)
                S = pair * nt + jb - SHIFT
                if S >= 0:
                    mm2_slot(states[S // nt], S % nt)
                else:
                    # keep the PE clock gate warm through the fill phase
                    pe_filler(6)
        for S in range(heads // 2 * nt - SHIFT, heads // 2 * nt):
            mm2_slot(states[S // nt], S % nt, tail=True)
            pe_filler(4)

    _split_multi_waits(nc, mybir)
    return nc


def _get_nc():
    if "nc" not in _cache:
        _cache["nc"] = _build_nc()
    return _cache["nc"]


def _run(q, k, v, trace=False):
    from concourse.bass_utils import run_bass_kernel_spmd

    b, heads, h, w, d = 8, 8, 32, 32, 64
    q = np.ascontiguousarray(np.asarray(q, dtype=np.float32))
    k = np.ascontiguousarray(np.asarray(k, dtype=np.float32))
    v = np.ascontiguousarray(np.asarray(v, dtype=np.float32))
    assert q.shape == (b, heads, h, w, d), q.shape

    nc = _get_nc()
    in_maps = [
        {
            "q": q[c].reshape(heads, h * w, d),
            "k": k[c].reshape(heads, h * w, d),
            "v": v[c].reshape(heads, h * w, d),
        }
        for c in range(b)
    ]
    res = run_bass_kernel_spmd(nc, in_maps, core_ids=list(range(b)), trace=trace)
    out = np.stack(
        [res.results[c]["out"].reshape(h, w, heads * d) for c in range(b)]
    )
    return out, res


def kernel(q, k, v):
    out, _ = _run(q, k, v)
    return out


# revision 34
# speedup vs baseline: 1.0147x; 1.0147x over previous
"""Trainium2 Bass kernel for multi-head quadratic spatial attention.

Problem: q,k,v [b=8, heads=8, h=32, w=32, d=64] fp32; full attention over
the 1024-position spatial grid independently per (b, head); output
[b, h, w, heads*d].

Sharding: data-parallel over batch — core c handles b=c (8 heads of
[1024, 64] attention per core), no cross-core communication.

Per-core pipeline (heads processed in PAIRS; matmuls bf16 with fp32 PSUM
accumulation). The PE executes serially on this toolchain, so the design
minimizes streamed columns + instruction count and keeps the HAM clock
gate warm (no transpose-heavy stretches > ~3.4us, dummy-matmul warm-up):
  - p-major seq tiling (seq = p*8 + t); ONE 4D casting DMA per (tensor,
    pair) interleaving the two heads -> 3 gpsimd triggers per pair
  - 40 dummy ident matmuls warm the PE clock gate (1.2 -> 2.4 GHz) while
    the first DMAs land
  - pair-interleaved natural tiles [128, t, 2, d]: one [128,128] PE
    transpose per block yields head A's d-rows on partitions 0:64 and
    B's on 64:128 — the packed pair layout mm1 wants
  - mm1 row-tiled: head A contracts on PE rows 0:64, head B on 64:128
    -> St [128, 1024] fp32 (separate tiles, freed by their own exp)
  - exp on ScalarE (activation Exp); optional per-(jb, head) offload to
    VectorE via the Schraudolph bit-trick (fused tensor_scalar
    mult+add -> int16 == bf16 exp approx) to unload the ScalarE
  - mm2: lhsT = [V | 1] j-chunk [128, 65] bf16, rhs = Pt slices ->
    accumulate PSUM Ot [65, 512] per i-half; row 64 = softmax sums
  - epilogue in bf16: ot copy on VectorE, PE transposes back (FWL), one
    batched reciprocal [128,4] + per-block tensor_scalar normalize into
    fp32 ostage, stores on the sync HWDGE ring
"""

from contextlib import ExitStack

import numpy as np

F32 = None
BF16 = None
I16 = None

_cache = {}

# Schraudolph exp in bf16 bit-space: bf16_bits(exp(s*x)) ~= round(x*A + B)
# A = s * 2^7/ln2, B = 2^7*(127 - sigma), sigma = 0.0430 balances the
# piecewise-linear 2^frac error (max rel err ~3%, mostly cancelling in the
# softmax average).
SCALE = 64.0 ** -0.5
SCHRAUD_A = SCALE * 128.0 / float(np.log(2.0))
SCHRAUD_B = 128.0 * (127.0 - 0.0430)

N_WARM = 20  # dummy matmuls to flip the PE HAM clock gate before real work

# (jb, head-in-pair) St tiles exp'd on VectorE via the Schraudolph bit-trick
# instead of ScalarE's exact exp. Each entry moves 1/16 of the attention
# weight mass to a ~3% weight-error approximation (mostly cancelling in the
# softmax average) and takes ~1.1us/pair off the ScalarE critical path.
SCHRAUD_TILES = frozenset()


def _imports():
    global F32, BF16, I16
    import concourse.bass as bass
    import concourse.tile as tile
    from concourse import mybir
    from concourse.masks import make_identity

    F32 = mybir.dt.float32
    BF16 = mybir.dt.bfloat16
    I16 = mybir.dt.int16
    return bass, tile, mybir, make_identity


def _split_multi_waits(nc, mybir):
    """Walrus in this container supports only ONE sync-wait per instruction.
    Hoist extra waits onto same-engine InstNoOp's inserted just before."""
    ctr = 0
    for f in nc.m.functions:
        for bb in f.blocks:
            insts = bb.instructions
            if not any(
                i.sync_info and i.sync_info.on_wait and len(i.sync_info.on_wait) > 1
                for i in insts
            ):
                continue
            out = []
            for inst in insts:
                si = inst.sync_info
                waits = list(si.on_wait) if si and si.on_wait else []
                if len(waits) > 1:
                    for w in waits[:-1]:
                        ctr += 1
                        nop = mybir.InstNoOp(
                            name=f"I-wsplit-{ctr}",
                            engine=inst.engine,
                            ins=[],
                            outs=[],
                            sync_info=mybir.SyncInfo(on_wait=[w], on_update=[]),
                        )
                        nc.register_instruction(nop)
                        out.append(nop)
                    si.on_wait = waits[-1:]
                out.append(inst)
            bb.instructions = out


def _build_nc(heads=8, seq=1024, d=64):
    bass, tile, mybir, make_identity = _imports()
    assert heads % 2 == 0 and seq == 1024 and d == 64
    nt = seq // 128          # 8 blocks of 128 positions
    nh = seq // 512          # 2 i-halves of 512
    dv = d + 1
    TS_MULT = mybir.AluOpType.mult
    TS_ADD = mybir.AluOpType.add

    nc = bass.Bass(trn_type="TRN2", target_bir_lowering=False)
    q_d = nc.dram_tensor("q", [heads, seq, d], F32, kind="ExternalInput")
    k_d = nc.dram_tensor("k", [heads, seq, d], F32, kind="ExternalInput")
    v_d = nc.dram_tensor("v", [heads, seq, d], F32, kind="ExternalInput")
    o_d = nc.dram_tensor("out", [seq, heads * d], F32, kind="ExternalOutput")

    # p-major: seq = p*nt + t; per-(p, t) HBM runs are 256B contiguous
    q_ap = q_d[:].rearrange("n (p t) d -> n p t d", p=128)
    k_ap = k_d[:].rearrange("n (p t) d -> n p t d", p=128)
    v_ap = v_d[:].rearrange("n (p t) d -> n p t d", p=128)
    o_ap = o_d[:].rearrange("(p t) c -> p t c", p=128)

    with tile.TileContext(nc) as tc, ExitStack() as ctx:
        consts = ctx.enter_context(tc.tile_pool(name="consts", bufs=1))
        nat = ctx.enter_context(tc.tile_pool(name="nat", bufs=2))
        dmaj = ctx.enter_context(tc.tile_pool(name="dmaj", bufs=2))
        ptp = ctx.enter_context(tc.tile_pool(name="ptp", bufs=36))
        otp = ctx.enter_context(tc.tile_pool(name="otp", bufs=3))
        outp = ctx.enter_context(tc.tile_pool(name="outp", bufs=3))
        small = ctx.enter_context(tc.tile_pool(name="small", bufs=4))

        # PSUM banks: st 2x2 (0-3) + oacc/ob/warm 2x1 (4-5) + tp 2x1 (6-7)
        st_ps = ctx.enter_context(tc.tile_pool(name="st_ps", bufs=2, space="PSUM"))
        oa_ps = ctx.enter_context(tc.tile_pool(name="oa_ps", bufs=2, space="PSUM"))
        tp_ps = ctx.enter_context(tc.tile_pool(name="tp_ps", bufs=2, space="PSUM"))

        ident_bf = consts.tile([128, 128], BF16)
        make_identity(nc, ident_bf[:])

        # Warm-up / filler matmuls keep the PE HAM clock gate at 2.4 GHz:
        # an idle (or transpose-only) stretch > ~3.4us re-throttles the PE
        # clock to 1.2 GHz for the next several microseconds. wsrc is
        # memset-ready within ~200ns of kernel start. N=512 streams give
        # ~213ns of HAM-counted busy per filler instruction.
        wsrc = consts.tile([128, 512], BF16)
        nc.vector.memset(wsrc[:], 0.25)

        def pe_filler(n):
            # fresh tile per burst: fillers WAR-chain only onto transient
            # transpose tiles, never onto live oacc accumulators
            t = tp_ps.tile([128, 512], F32, tag="tp", name="warm")
            for _ in range(n):
                nc.tensor.matmul(
                    t[:], wsrc[:, 0:128], wsrc[:], start=True, stop=True
                )

        pe_filler(N_WARM)

        def load_and_transpose(pair):
            """DMA pair inputs (bf16 cast, one 4D DMA per tensor) and build
            packed d-major tiles: head A on partitions 0:64, head B on
            64:128 (one [128,128] PE transpose per block)."""
            st8 = {"heads": (2 * pair, 2 * pair + 1), "v": None, "pts": [],
                   "oacc": {}, "ostage": {}}
            # pair-interleaved natural tiles: [..., 2, d] with head A at
            # index 0 and head B at 1, so one [128, 128] PE transpose of a
            # block yields A's d-rows on partitions 0:64 and B's on 64:128.
            qp = nat.tile([128, nt, 2, d], BF16, tag="qp")
            kp = nat.tile([128, nt, 2, d], BF16, tag="kp")
            hh = nt // 2
            if pair == 0:
                # halved loads, first halves of BOTH heads first, so the
                # first transpose group (blocks 0-3, pair-interleaved) can
                # start after ~2 trigger slots instead of ~6
                for lo, hi in ((0, hh), (hh, nt)):
                    for src_ap, dst in ((q_ap, qp), (k_ap, kp)):
                        for idx, n in enumerate(st8["heads"]):
                            nc.gpsimd.dma_start(
                                out=dst[:, lo:hi, idx, :], in_=src_ap[n, :, lo:hi]
                            )
            else:
                for idx, n in enumerate(st8["heads"]):
                    nc.gpsimd.dma_start(out=qp[:, :, idx, :], in_=q_ap[n])
                    nc.gpsimd.dma_start(out=kp[:, :, idx, :], in_=k_ap[n])
            vp = nat.tile([128, nt, 2, dv], BF16, tag="vp")
            # ones columns for the softmax-denominator trick
            nc.vector.memset(vp[:, :, :, d : d + 1], 1.0)
            for idx, n in enumerate(st8["heads"]):
                nc.gpsimd.dma_start(out=vp[:, :, idx, 0:d], in_=v_ap[n])
            st8["v"] = vp
            qt = dmaj.tile([128, seq], BF16, tag="qt")
            kt = dmaj.tile([128, seq], BF16, tag="kt")
            for g in range(nt // 4):
                for src, dst in ((qp, qt), (kp, kt)):
                    tp = tp_ps.tile([128, 512], BF16, tag="tp")
                    for u in range(4):
                        t = g * 4 + u
                        nc.tensor.transpose(
                            tp[:, u * 128 : (u + 1) * 128],
                            src[:, t, :, :],
                            ident_bf[:],
                        )
                    nc.vector.tensor_copy(
                        out=dst[:, g * 512 : (g + 1) * 512], in_=tp[:]
                    )
                    if pair == 0:
                        # PE is otherwise DMA-bound here; keep the clock warm
                        pe_filler(2)
            st8["qt"], st8["kt"] = qt, kt
            return st8

        def mm1_exp(s, jb):
            """Row-tiled pair mm1 into per-head St tiles + per-head exp.
            Separate St tiles mean head A's tile is released as soon as its
            own exp finishes."""
            qt, kt = s["qt"], s["kt"]
            stA = st_ps.tile([128, seq], F32, name="stA", tag="st")
            stB = st_ps.tile([128, seq], F32, name="stB", tag="st")
            for c in range(nh):
                nc.tensor.matmul(
                    stA[:, c * 512 : (c + 1) * 512],
                    kt[0:64, jb * 128 : (jb + 1) * 128],
                    qt[0:64, c * 512 : (c + 1) * 512],
                    start=True,
                    stop=True,
                )
                nc.tensor.matmul(
                    stB[:, c * 512 : (c + 1) * 512],
                    kt[64:128, jb * 128 : (jb + 1) * 128],
                    qt[64:128, c * 512 : (c + 1) * 512],
                    start=True,
                    stop=True,
                )
            for idx, st in enumerate((stA, stB)):
                pt = ptp.tile([128, seq], BF16, name="pt", tag="pt")
                if (jb, idx) in SCHRAUD_TILES:
                    nc.vector.tensor_scalar(
                        out=pt[:].bitcast(I16),
                        in0=st[:],
                        scalar1=SCHRAUD_A,
                        scalar2=SCHRAUD_B,
                        op0=TS_MULT,
                        op1=TS_ADD,
                    )
                else:
                    nc.scalar.activation(
                        out=pt[:],
                        in_=st[:],
                        func=mybir.ActivationFunctionType.Exp,
                        scale=SCALE,
                    )
                s["pts"].append((jb, idx, pt))

        # slot s -> (group, phase); phase-1 slots sit at s>=2 so they only
        # run once all 8 jbs' Pt tiles exist (6-jb-shifted pipeline), while
        # at most 2 groups' oacc accumulators are ever live.
        SLOT_ORDER = [(0, 0), (1, 0), (0, 1), (1, 1), (2, 0), (3, 0), (2, 1), (3, 1)]

        def mm2_slot(s, slot, tail=False):
            """One PE-stream slot of the pair's mm2: 4 accumulating
            matmuls of a (head, half) group; epilogue on the closing
            phase."""
            g, phase = SLOT_ORDER[slot]
            idx, half = g // 2, g % 2
            if phase == 0:
                s["oacc"][g] = oa_ps.tile([dv, 512], F32, name="oacc", tag="oacc")
            oacc = s["oacc"][g]
            off = half * 512
            for jj in range(4):
                jb = phase * 4 + jj
                jb2, idx2, pt = s["pts"][jb * 2 + idx]
                assert jb2 == jb and idx2 == idx
                nc.tensor.matmul(
                    oacc[:],
                    s["v"][:, jb, idx, :],
                    pt[:, off : off + 512],
                    start=(jb == 0),
                    stop=(jb == nt - 1),
                )
            if phase == 1:
                _epilogue(s, idx, half, oacc, tail=tail)

        def _epilogue(s, idx, half, oacc, tail=False):
            n = s["heads"][idx]
            if idx not in s["ostage"]:
                s["ostage"][idx] = outp.tile(
                    [128, nt, d], F32, name="ostage", tag="ostage"
                )
            ostage = s["ostage"][idx]
            ot = otp.tile([dv, 512], BF16, tag="ot")
            if tail:
                # ScalarE is idle after its last exp; shorten the tail chain
                nc.scalar.copy(out=ot[:], in_=oacc[:])
            else:
                nc.vector.tensor_copy(out=ot[:], in_=oacc[:])
            # ob shares the oacc pool banks: rotation interleaves
            # oacc(g) -> ob(g) -> oacc(g+1), each WAR-safe by then.
            # dv+1 padding keeps each transpose's PSUM write 4B-aligned.
            ob = oa_ps.tile([128, 4, dv + 1], BF16, tag="oacc")
            for u in range(4):
                nc.tensor.transpose(
                    ob[:, u, 0:dv],
                    ot[:, u * 128 : (u + 1) * 128],
                    ident_bf[0:dv, 0:dv],
                )
            rec = small.tile([128, 4], F32, tag="rec")
            nc.vector.reciprocal(out=rec[:], in_=ob[:, :, d])
            for u in range(4):
                t = half * 4 + u
                nc.vector.tensor_scalar_mul(
                    ostage[:, t, :], ob[:, u, 0:d], rec[:, u : u + 1]
                )
            if half == nh - 1:
                nc.sync.dma_start(
                    out=o_ap[:, :, n * d : (n + 1) * d], in_=ostage[:]
                )

        # software pipeline, 6-jb shifted: mm2 slot S runs alongside mm1 of
        # global jb S+6, so pair 0's loop is mm2-dense from jb 6 (instead
        # of mm1-only for a whole pair) and the mm2-only tail is 6 slots.
        SHIFT = 6
        states = []
        for pair in range(heads // 2):
            cur = load_and_transpose(pair)
            states.append(cur)
            for jb in range(nt):
                mm1_exp(cur, jb)
                S = pair * nt + jb - SHIFT
                if S >= 0:
                    mm2_slot(states[S // nt], S % nt)
                else:
                    # keep the PE clock gate warm through the fill phase
                    pe_filler(5)
        for S in range(heads // 2 * nt - SHIFT, heads // 2 * nt):
            mm2_slot(states[S // nt], S % nt, tail=True)
            pe_filler(3)

    _split_multi_waits(nc, mybir)
    return nc


def _get_nc():
    if "nc" not in _cache:
        _cache["nc"] = _build_nc()
    return _cache["nc"]


def _run(q, k, v, trace=False):
    from concourse.bass_utils import run_bass_kernel_spmd

    b, heads, h, w, d = 8, 8, 32, 32, 64
    q = np.ascontiguousarray(np.asarray(q, dtype=np.float32))
    k = np.ascontiguousarray(np.asarray(k, dtype=np.float32))
    v = np.ascontiguousarray(np.asarray(v, dtype=np.float32))
    assert q.shape == (b, heads, h, w, d), q.shape

    nc = _get_nc()
    in_maps = [
        {
            "q": q[c].reshape(heads, h * w, d),
            "k": k[c].reshape(heads, h * w, d),
            "v": v[c].reshape(heads, h * w, d),
        }
        for c in range(b)
    ]
    res = run_bass_kernel_spmd(nc, in_maps, core_ids=list(range(b)), trace=trace)
    out = np.stack(
        [res.results[c]["out"].reshape(h, w, heads * d) for c in range(b)]
    )
    return out, res


def kernel(q, k, v):
    out, _ = _run(q, k, v)
    return out


# revision 36
# speedup vs baseline: 1.0727x; 1.0572x over previous
"""Trainium2 Bass kernel for multi-head quadratic spatial attention.

Problem: q,k,v [b=8, heads=8, h=32, w=32, d=64] fp32; full attention over
the 1024-position spatial grid independently per (b, head); output
[b, h, w, heads*d].

Sharding: data-parallel over batch — core c handles b=c (8 heads of
[1024, 64] attention per core), no cross-core communication.

Per-core pipeline (heads processed in PAIRS; matmuls bf16 with fp32 PSUM
accumulation). The PE executes serially on this toolchain, so the design
minimizes streamed columns + instruction count and keeps the HAM clock
gate warm (no transpose-heavy stretches > ~3.4us, dummy-matmul warm-up):
  - p-major seq tiling (seq = p*8 + t); ONE 4D casting DMA per (tensor,
    pair) interleaving the two heads -> 3 gpsimd triggers per pair
  - 40 dummy ident matmuls warm the PE clock gate (1.2 -> 2.4 GHz) while
    the first DMAs land
  - pair-interleaved natural tiles [128, t, 2, d]: one [128,128] PE
    transpose per block yields head A's d-rows on partitions 0:64 and
    B's on 64:128 — the packed pair layout mm1 wants
  - mm1 row-tiled: head A contracts on PE rows 0:64, head B on 64:128
    -> St [128, 1024] fp32 (separate tiles, freed by their own exp)
  - exp on ScalarE (activation Exp); optional per-(jb, head) offload to
    VectorE via the Schraudolph bit-trick (fused tensor_scalar
    mult+add -> int16 == bf16 exp approx) to unload the ScalarE
  - mm2: lhsT = [V | 1] j-chunk [128, 65] bf16, rhs = Pt slices ->
    accumulate PSUM Ot [65, 512] per i-half; row 64 = softmax sums
  - epilogue in bf16: ot copy on VectorE, PE transposes back (FWL), one
    batched reciprocal [128,4] + per-block tensor_scalar normalize into
    fp32 ostage, stores on the sync HWDGE ring
"""

from contextlib import ExitStack

import numpy as np

F32 = None
BF16 = None
I16 = None

_cache = {}

# Schraudolph exp in bf16 bit-space: bf16_bits(exp(s*x)) ~= round(x*A + B)
# A = s * 2^7/ln2, B = 2^7*(127 - sigma), sigma = 0.0430 balances the
# piecewise-linear 2^frac error (max rel err ~3%, mostly cancelling in the
# softmax average).
SCALE = 64.0 ** -0.5
SCHRAUD_A = SCALE * 128.0 / float(np.log(2.0))
SCHRAUD_B = 128.0 * (127.0 - 0.0430)

N_WARM = 20  # dummy matmuls to flip the PE HAM clock gate before real work

# (jb, head-in-pair) St tiles exp'd on VectorE via the Schraudolph bit-trick
# instead of ScalarE's exact exp. Each entry moves 1/16 of the attention
# weight mass to a ~3% weight-error approximation (mostly cancelling in the
# softmax average) and takes ~1.1us/pair off the ScalarE critical path.
SCHRAUD_TILES = frozenset()


def _imports():
    global F32, BF16, I16
    import concourse.bass as bass
    import concourse.tile as tile
    from concourse import mybir
    from concourse.masks import make_identity

    F32 = mybir.dt.float32
    BF16 = mybir.dt.bfloat16
    I16 = mybir.dt.int16
    return bass, tile, mybir, make_identity


def _split_multi_waits(nc, mybir):
    """Walrus in this container supports only ONE sync-wait per instruction.
    Hoist extra waits onto same-engine InstNoOp's inserted just before."""
    ctr = 0
    for f in nc.m.functions:
        for bb in f.blocks:
            insts = bb.instructions
            if not any(
                i.sync_info and i.sync_info.on_wait and len(i.sync_info.on_wait) > 1
                for i in insts
            ):
                continue
            out = []
            for inst in insts:
                si = inst.sync_info
                waits = list(si.on_wait) if si and si.on_wait else []
                if len(waits) > 1:
                    for w in waits[:-1]:
                        ctr += 1
                        nop = mybir.InstNoOp(
                            name=f"I-wsplit-{ctr}",
                            engine=inst.engine,
                            ins=[],
                            outs=[],
                            sync_info=mybir.SyncInfo(on_wait=[w], on_update=[]),
                        )
                        nc.register_instruction(nop)
                        out.append(nop)
                    si.on_wait = waits[-1:]
                out.append(inst)
            bb.instructions = out


def _build_nc(heads=8, seq=1024, d=64):
    bass, tile, mybir, make_identity = _imports()
    assert heads % 2 == 0 and seq == 1024 and d == 64
    nt = seq // 128          # 8 blocks of 128 positions
    nh = seq // 512          # 2 i-halves of 512
    dv = d + 1
    TS_MULT = mybir.AluOpType.mult
    TS_ADD = mybir.AluOpType.add

    nc = bass.Bass(trn_type="TRN2", target_bir_lowering=False)
    q_d = nc.dram_tensor("q", [heads, seq, d], F32, kind="ExternalInput")
    k_d = nc.dram_tensor("k", [heads, seq, d], F32, kind="ExternalInput")
    v_d = nc.dram_tensor("v", [heads, seq, d], F32, kind="ExternalInput")
    o_d = nc.dram_tensor("out", [seq, heads * d], F32, kind="ExternalOutput")

    # p-major: seq = p*nt + t; per-(p, t) HBM runs are 256B contiguous
    q_ap = q_d[:].rearrange("n (p t) d -> n p t d", p=128)
    k_ap = k_d[:].rearrange("n (p t) d -> n p t d", p=128)
    v_ap = v_d[:].rearrange("n (p t) d -> n p t d", p=128)
    o_ap = o_d[:].rearrange("(p t) c -> p t c", p=128)

    with tile.TileContext(nc) as tc, ExitStack() as ctx:
        consts = ctx.enter_context(tc.tile_pool(name="consts", bufs=1))
        nat = ctx.enter_context(tc.tile_pool(name="nat", bufs=2))
        dmaj = ctx.enter_context(tc.tile_pool(name="dmaj", bufs=2))
        ptp = ctx.enter_context(tc.tile_pool(name="ptp", bufs=36))
        otp = ctx.enter_context(tc.tile_pool(name="otp", bufs=3))
        outp = ctx.enter_context(tc.tile_pool(name="outp", bufs=3))
        small = ctx.enter_context(tc.tile_pool(name="small", bufs=4))

        # PSUM banks: st 2x2 (0-3) + oacc/ob/warm 2x1 (4-5) + tp 2x1 (6-7)
        st_ps = ctx.enter_context(tc.tile_pool(name="st_ps", bufs=2, space="PSUM"))
        oa_ps = ctx.enter_context(tc.tile_pool(name="oa_ps", bufs=2, space="PSUM"))
        tp_ps = ctx.enter_context(tc.tile_pool(name="tp_ps", bufs=2, space="PSUM"))

        ident_bf = consts.tile([128, 128], BF16)
        make_identity(nc, ident_bf[:])

        # Warm-up / filler matmuls keep the PE HAM clock gate at 2.4 GHz:
        # an idle (or transpose-only) stretch > ~3.4us re-throttles the PE
        # clock to 1.2 GHz for the next several microseconds. wsrc is
        # memset-ready within ~200ns of kernel start. N=512 streams give
        # ~213ns of HAM-counted busy per filler instruction.
        wsrc = consts.tile([128, 512], BF16)
        nc.vector.memset(wsrc[:], 0.25)

        def pe_filler(n):
            # fresh tile per burst: fillers WAR-chain only onto transient
            # transpose tiles, never onto live oacc accumulators
            t = tp_ps.tile([128, 512], F32, tag="tp", name="warm")
            for _ in range(n):
                nc.tensor.matmul(
                    t[:], wsrc[:, 0:128], wsrc[:], start=True, stop=True
                )

        pe_filler(N_WARM)

        def load_and_transpose(pair):
            """DMA pair inputs (bf16 cast, one 4D DMA per tensor) and build
            packed d-major tiles: head A on partitions 0:64, head B on
            64:128 (one [128,128] PE transpose per block)."""
            st8 = {"heads": (2 * pair, 2 * pair + 1), "v": None, "pts": [],
                   "oacc": {}, "ostage": {}}
            # pair-interleaved natural tiles: [..., 2, d] with head A at
            # index 0 and head B at 1, so one [128, 128] PE transpose of a
            # block yields A's d-rows on partitions 0:64 and B's on 64:128.
            qp = nat.tile([128, nt, 2, d], BF16, tag="qp")
            kp = nat.tile([128, nt, 2, d], BF16, tag="kp")
            hh = nt // 2
            if pair == 0:
                # halved loads, first halves of BOTH heads first, so the
                # first transpose group (blocks 0-3, pair-interleaved) can
                # start after ~2 trigger slots instead of ~6
                for lo, hi in ((0, hh), (hh, nt)):
                    for src_ap, dst in ((q_ap, qp), (k_ap, kp)):
                        for idx, n in enumerate(st8["heads"]):
                            nc.gpsimd.dma_start(
                                out=dst[:, lo:hi, idx, :], in_=src_ap[n, :, lo:hi]
                            )
            else:
                for idx, n in enumerate(st8["heads"]):
                    nc.gpsimd.dma_start(out=qp[:, :, idx, :], in_=q_ap[n])
                    nc.gpsimd.dma_start(out=kp[:, :, idx, :], in_=k_ap[n])
            vp = nat.tile([128, nt, 2, dv], BF16, tag="vp")
            # ones columns for the softmax-denominator trick
            nc.vector.memset(vp[:, :, :, d : d + 1], 1.0)
            for idx, n in enumerate(st8["heads"]):
                nc.gpsimd.dma_start(out=vp[:, :, idx, 0:d], in_=v_ap[n])
            st8["v"] = vp
            qt = dmaj.tile([128, seq], BF16, tag="qt")
            kt = dmaj.tile([128, seq], BF16, tag="kt")
            for g in range(nt // 4):
                for src, dst in ((qp, qt), (kp, kt)):
                    tp = tp_ps.tile([128, 512], BF16, tag="tp")
                    for u in range(4):
                        t = g * 4 + u
                        nc.tensor.transpose(
                            tp[:, u * 128 : (u + 1) * 128],
                            src[:, t, :, :],
                            ident_bf[:],
                        )
                    nc.vector.tensor_copy(
                        out=dst[:, g * 512 : (g + 1) * 512], in_=tp[:]
                    )
                    if pair == 0:
                        # PE is otherwise DMA-bound here; keep the clock warm
                        pe_filler(1)
            st8["qt"], st8["kt"] = qt, kt
            return st8

        def mm1_exp(s, jb):
            """Row-tiled pair mm1 into per-head St tiles + per-head exp.
            Separate St tiles mean head A's tile is released as soon as its
            own exp finishes."""
            qt, kt = s["qt"], s["kt"]
            stA = st_ps.tile([128, seq], F32, name="stA", tag="st")
            stB = st_ps.tile([128, seq], F32, name="stB", tag="st")
            for c in range(nh):
                nc.tensor.matmul(
                    stA[:, c * 512 : (c + 1) * 512],
                    kt[0:64, jb * 128 : (jb + 1) * 128],
                    qt[0:64, c * 512 : (c + 1) * 512],
                    start=True,
                    stop=True,
                )
                nc.tensor.matmul(
                    stB[:, c * 512 : (c + 1) * 512],
                    kt[64:128, jb * 128 : (jb + 1) * 128],
                    qt[64:128, c * 512 : (c + 1) * 512],
                    start=True,
                    stop=True,
                )
            for idx, st in enumerate((stA, stB)):
                pt = ptp.tile([128, seq], BF16, name="pt", tag="pt")
                if (jb, idx) in SCHRAUD_TILES:
                    nc.vector.tensor_scalar(
                        out=pt[:].bitcast(I16),
                        in0=st[:],
                        scalar1=SCHRAUD_A,
                        scalar2=SCHRAUD_B,
                        op0=TS_MULT,
                        op1=TS_ADD,
                    )
                else:
                    nc.scalar.activation(
                        out=pt[:],
                        in_=st[:],
                        func=mybir.ActivationFunctionType.Exp,
                        scale=SCALE,
                    )
                s["pts"].append((jb, idx, pt))

        # slot s -> (group, phase); phase-1 slots sit at s>=2 so they only
        # run once all 8 jbs' Pt tiles exist (6-jb-shifted pipeline), while
        # at most 2 groups' oacc accumulators are ever live.
        SLOT_ORDER = [(0, 0), (1, 0), (0, 1), (1, 1), (2, 0), (3, 0), (2, 1), (3, 1)]

        def mm2_slot(s, slot, tail=False):
            """One PE-stream slot of the pair's mm2: 4 accumulating
            matmuls of a (head, half) group; epilogue on the closing
            phase."""
            g, phase = SLOT_ORDER[slot]
            idx, half = g // 2, g % 2
            if phase == 0:
                s["oacc"][g] = oa_ps.tile([dv, 512], F32, name="oacc", tag="oacc")
            oacc = s["oacc"][g]
            off = half * 512
            for jj in range(4):
                jb = phase * 4 + jj
                jb2, idx2, pt = s["pts"][jb * 2 + idx]
                assert jb2 == jb and idx2 == idx
                nc.tensor.matmul(
                    oacc[:],
                    s["v"][:, jb, idx, :],
                    pt[:, off : off + 512],
                    start=(jb == 0),
                    stop=(jb == nt - 1),
                )
            if phase == 1:
                _epilogue(s, idx, half, oacc, tail=tail)

        def _epilogue(s, idx, half, oacc, tail=False):
            n = s["heads"][idx]
            if idx not in s["ostage"]:
                s["ostage"][idx] = outp.tile(
                    [128, nt, d], F32, name="ostage", tag="ostage"
                )
            ostage = s["ostage"][idx]
            ot = otp.tile([dv, 512], BF16, tag="ot")
            if tail:
                # ScalarE is idle after its last exp; shorten the tail chain
                nc.scalar.copy(out=ot[:], in_=oacc[:])
            else:
                nc.vector.tensor_copy(out=ot[:], in_=oacc[:])
            # ob shares the oacc pool banks: rotation interleaves
            # oacc(g) -> ob(g) -> oacc(g+1), each WAR-safe by then.
            # dv+1 padding keeps each transpose's PSUM write 4B-aligned.
            ob = oa_ps.tile([128, 4, dv + 1], BF16, tag="oacc")
            for u in range(4):
                nc.tensor.transpose(
                    ob[:, u, 0:dv],
                    ot[:, u * 128 : (u + 1) * 128],
                    ident_bf[0:dv, 0:dv],
                )
            rec = small.tile([128, 4], F32, tag="rec")
            nc.vector.reciprocal(out=rec[:], in_=ob[:, :, d])
            for u in range(4):
                t = half * 4 + u
                nc.vector.tensor_scalar_mul(
                    ostage[:, t, :], ob[:, u, 0:d], rec[:, u : u + 1]
                )
            if half == nh - 1:
                nc.sync.dma_start(
                    out=o_ap[:, :, n * d : (n + 1) * d], in_=ostage[:]
                )

        # software pipeline, 6-jb shifted: mm2 slot S runs alongside mm1 of
        # global jb S+6, so pair 0's loop is mm2-dense from jb 6 (instead
        # of mm1-only for a whole pair) and the mm2-only tail is 6 slots.
        SHIFT = 6
        states = []
        for pair in range(heads // 2):
            cur = load_and_transpose(pair)
            states.append(cur)
            for jb in range(nt):
                mm1_exp(cur, jb)
                S = pair * nt + jb - SHIFT
                if S >= 0:
                    mm2_slot(states[S // nt], S % nt)
                else:
                    # keep the PE clock gate warm through the fill phase
                    pe_filler(3)
        for S in range(heads // 2 * nt - SHIFT, heads // 2 * nt):
            mm2_slot(states[S // nt], S % nt, tail=True)

    _split_multi_waits(nc, mybir)
    return nc


def _get_nc():
    if "nc" not in _cache:
        _cache["nc"] = _build_nc()
    return _cache["nc"]


def _run(q, k, v, trace=False):
    from concourse.bass_utils import run_bass_kernel_spmd

    b, heads, h, w, d = 8, 8, 32, 32, 64
    q = np.ascontiguousarray(np.asarray(q, dtype=np.float32))
    k = np.ascontiguousarray(np.asarray(k, dtype=np.float32))
    v = np.ascontiguousarray(np.asarray(v, dtype=np.float32))
    assert q.shape == (b, heads, h, w, d), q.shape

    nc = _get_nc()
    in_maps = [
        {
            "q": q[c].reshape(heads, h * w, d),
            "k": k[c].reshape(heads, h * w, d),
            "v": v[c].reshape(heads, h * w, d),
        }
        for c in range(b)
    ]
    res = run_bass_kernel_spmd(nc, in_maps, core_ids=list(range(b)), trace=trace)
    out = np.stack(
        [res.results[c]["out"].reshape(h, w, heads * d) for c in range(b)]
    )
    return out, res


def kernel(q, k, v):
    out, _ = _run(q, k, v)
    return out
